# revision 7
# baseline (speedup 1.0000x reference)
"""Trainium2 Bass kernel for nn_GRNNTransformGated (recursive tree GRNN over
1024 independent 10-level binary jets).

Strategy (v2):
  - Data-parallel over jets: 8 cores x 128 trees each.
  - BIT-REVERSED per-tree node layout: storing level j in bit-reversed local
    order makes the two children of parent position q land at positions q and
    q + 2^j of the level below -- every child "gather" becomes two contiguous
    stride-1 slices, so all DVE ops run in packed bf16 2x mode.
  - Softmax shift-invariance: z gates computed as differences d_m = z_m - z_3,
    so the z matmul is 4Hx3H (12 matmuls) instead of 4Hx4H (16), only 3 exps,
    e3 == 1 (numerator gets +u, denominator gets +1).
  - Feature-major layout [128 channels (partitions), nodes (free)]; weight
    blocks stationary, 25 matmuls per 512-node tile.
  - 8 separate PSUM banks (pu, pr0-2, ph, pz0-2) so consecutive tiles overlap
    with only early-read WAR dependencies -- keeps the PE dense and the HAM
    clock-gate warm (K=8/8).
  - conv_chain collapses: for w>0, b>=0, f(f(f(x))) = w^2*relu(w*x+b) + (w*b+b).
  - sigmoid via tanh; the 0.5 is folded into W_h on the host.
  - Per-tree interleaved temporaries (t01, e12, p12) let pairs of elementwise
    ops fuse into single FD=1024 stride-1 instructions.
"""

import sys

for _p in ("/opt/trn_rl_repo", "/root/.axon_site/_ro/trn_rl_repo"):
    if _p not in sys.path:
        sys.path.insert(0, _p)

import numpy as np

B = 1024
L = 10
H = 128
FEAT = 7
NCORES = 8
TPC = B // NCORES          # trees per core = 128
TCH = 16                   # trees per chunk
NCHUNK = TPC // TCH        # 8 chunks
NPC = TPC * (2 ** L - 1)   # nodes per core = 130944
LOFF = [TPC * (2 ** j - 1) for j in range(L + 1)]  # level offsets in per-core ct
LEVEL_SIZES = [B * 2 ** j for j in range(L)]
OFF = np.concatenate([[0], np.cumsum(LEVEL_SIZES)]).astype(int)
INNER = LEVEL_SIZES[:-1]
COFF = np.concatenate([[0], np.cumsum(INNER)]).astype(int)

MMT = 512  # matmul node-tile size

_CACHE = {}


def _children_canonical(children):
    for j in range(L - 1):
        n = INNER[j]
        blk = children[COFF[j]:COFF[j + 1]]
        base = 2 * np.arange(n, dtype=np.int64)
        if not (np.array_equal(blk[:, 0], base) and np.array_equal(blk[:, 1], base + 1)):
            return False
    return True


def _numpy_fallback(contents, children, W_u, b_u, W_h, b_h, W_z, b_z, W_r, b_r,
                    conv_w, conv_b):
    w, b = float(conv_w[0]), float(conv_b[0])

    def conv_chain(x):
        for _ in range(3):
            x = np.maximum(w * x + b, 0.0)
        return x

    def sigmoid(x):
        return 1.0 / (1.0 + np.exp(-x))

    emb = None
    for j in reversed(range(L)):
        c = contents[OFF[j]:OFF[j + 1]]
        u = conv_chain(c @ W_u + b_u)
        if j == L - 1:
            emb = u
            continue
        ch = children[COFF[j]:COFF[j + 1]]
        h_L = emb[ch[:, 0]]
        h_R = emb[ch[:, 1]]
        hhu = np.concatenate([h_L, h_R, u], axis=1)
        r = sigmoid(hhu @ W_r + b_r)
        h_H = conv_chain((r * hhu) @ W_h + b_h)
        z = np.concatenate([h_H, hhu], axis=1) @ W_z + b_z
        zs = np.stack([z[:, :H], z[:, H:2 * H], z[:, 2 * H:3 * H], z[:, 3 * H:]], axis=-1)
        zs = zs - zs.max(axis=-1, keepdims=True)
        e = np.exp(zs)
        g = e / e.sum(axis=-1, keepdims=True)
        emb = g[..., 0] * h_H + g[..., 1] * h_L + g[..., 2] * h_R + g[..., 3] * u
    return emb.reshape(B, -1).astype(np.float32)


def _bitrev_perm(j):
    """perm[q] = bit-reverse of q over j bits."""
    if j == 0:
        return np.zeros(1, dtype=np.int64)
    return (
        np.arange(2 ** j, dtype=np.int64)
        .reshape((2,) * j)
        .transpose(tuple(reversed(range(j))))
        .ravel()
    )


def _build(cw, cb, collapsible, do_affine, A, C):
    from contextlib import ExitStack

    from concourse import bacc, bass, mybir, tile

    f32 = mybir.dt.float32
    bf16 = mybir.dt.bfloat16
    AF = mybir.ActivationFunctionType
    OP = mybir.AluOpType

    nc = bacc.Bacc()

    ct_d = nc.declare_dram_parameter("ct", [FEAT, NPC], bf16, isOutput=False)
    wu_d = nc.declare_dram_parameter("wu", [FEAT, H], bf16, isOutput=False)
    wr_d = nc.declare_dram_parameter("wr", [H, 3, 3, H], bf16, isOutput=False)
    wh_d = nc.declare_dram_parameter("wh", [H, 3, H], bf16, isOutput=False)
    wz_d = nc.declare_dram_parameter("wz", [H, 4, 3, H], bf16, isOutput=False)
    bv_d = nc.declare_dram_parameter("bvec", [H, 8], f32, isOutput=False)
    id_d = nc.declare_dram_parameter("ident", [H, H], f32, isOutput=False)
    out_d = nc.declare_dram_parameter("out", [TPC, H], f32, isOutput=True)

    with ExitStack() as ctx:
        tc = ctx.enter_context(tile.TileContext(nc))
        wpool = ctx.enter_context(tc.tile_pool(name="wts", bufs=1))
        epool = ctx.enter_context(tc.tile_pool(name="emb", bufs=1))
        ctpool = ctx.enter_context(tc.tile_pool(name="ct", bufs=3))
        spool = ctx.enter_context(tc.tile_pool(name="tmp", bufs=3))
        pp = ctx.enter_context(tc.tile_pool(name="ps", bufs=1, space="PSUM"))

        wu = wpool.tile([FEAT, H], bf16, tag="wu")
        wr = wpool.tile([H, 3, 3, H], bf16, tag="wr")
        wh = wpool.tile([H, 3, H], bf16, tag="wh")
        wz = wpool.tile([H, 4, 3, H], bf16, tag="wz")
        bv = wpool.tile([H, 8], f32, tag="bv")
        idt = wpool.tile([H, H], f32, tag="idt")
        nc.sync.dma_start(wu[:], wu_d[:])
        nc.sync.dma_start(wr[:], wr_d[:])
        nc.sync.dma_start(wh[:], wh_d[:])
        nc.sync.dma_start(wz[:], wz_d[:])
        nc.sync.dma_start(bv[:], bv_d[:])
        nc.sync.dma_start(idt[:], id_d[:])

        # emb level buffers (phase A holds one chunk; emb5 accumulates all chunks)
        e9 = epool.tile([H, TCH * 512], bf16, tag="e9")     # 8192
        e8 = epool.tile([H, TCH * 256], bf16, tag="e8")     # 4096
        e7 = epool.tile([H, TCH * 128], bf16, tag="e7")     # 2048
        e6 = epool.tile([H, TCH * 64], bf16, tag="e6")      # 1024
        emb5 = epool.tile([H, TPC * 32], bf16, tag="emb5")  # 4096 (all trees)

        zeros = wpool.tile([H, 2 * MMT], bf16, tag="zeros")
        nc.vector.memset(zeros[:], 0.0)

        pcnt = [0]  # global front-tile parity counter

        def conv_tail(dst):
            if collapsible:
                if do_affine:
                    nc.vector.tensor_scalar(dst, dst, A, C, OP.mult, OP.add)
            else:
                nc.scalar.activation(dst, dst, AF.Relu, bias=cb, scale=cw)
                nc.scalar.activation(dst, dst, AF.Relu, bias=cb, scale=cw)

        def front_tile(cbuf, w, s, n, ct_ap, half, pair):
            """Matmul/gate front-end for one 512-node tile. Writes u/hH/e0/e12
            into the pair-level buffers. `half` is 0/1 within the pair."""
            p = pcnt[0] % 2
            pcnt[0] += 1
            u2, hp2, e0p, e12p, s0 = pair
            ntt = n // w
            t0 = s // w
            tr = (s - s0) // w                              # tree offset in pair
            cb4 = cbuf.rearrange("p (t two w) -> p t two w", two=2, w=w)
            hL = cb4[:, t0:t0 + ntt, 0, :]                  # [H, ntt, w]
            hR = cb4[:, t0:t0 + ntt, 1, :]
            cb_both = cbuf[:, 2 * s:2 * s + 2 * n]          # [H, 2n] contiguous

            # ---- u ----
            pa = pp.tile([H, MMT], f32, name="pa", tag=f"pa{p}")
            nc.tensor.matmul(pa[:, :n], wu[:], ct_ap, start=True, stop=True)
            u = u2[:, half * MMT:half * MMT + n]
            nc.scalar.activation(u, pa[:, :n], AF.Relu, bias=bv[:, 0:1], scale=cw)
            conv_tail(u)
            # ---- r gates (as tanh), t0/t1 interleaved per tree ----
            pbs = [pp.tile([H, MMT], f32, name=f"pb{m}", tag=f"pb{m}_{p}")
                   for m in range(3)]
            rhs_k = [hL, hR, u]
            for m in range(3):
                for k in range(3):
                    nc.tensor.matmul(pbs[m][:, :n], wr[:, k, m, :], rhs_k[k],
                                     start=(k == 0), stop=(k == 2))
            t01 = spool.tile([H, 2 * MMT], bf16, name="t01", tag="t01", bufs=2)
            t01v = t01.rearrange("p (t two w) -> p t two w", two=2, w=w)
            t2 = spool.tile([H, MMT], bf16, name="t2", tag="t2", bufs=2)
            for m in range(2):
                nc.scalar.activation(t01v[:, :ntt, m, :], pbs[m][:, :n], AF.Tanh,
                                     bias=bv[:, 1 + m:2 + m], scale=0.5)
            nc.scalar.activation(t2[:, :n], pbs[2][:, :n], AF.Tanh,
                                 bias=bv[:, 3:4], scale=0.5)
            # ---- rh = (t+1) * hhu   (x0.5 folded into W_h) ----
            rh01 = spool.tile([H, 2 * MMT], bf16, name="rh01", tag="rh01", bufs=2)
            nc.vector.scalar_tensor_tensor(rh01[:, :2 * n], t01[:, :2 * n], 1.0,
                                           cb_both, OP.add, OP.mult)
            rh2 = spool.tile([H, MMT], bf16, name="rh2", tag="rh2", bufs=2)
            nc.vector.scalar_tensor_tensor(rh2[:, :n], t2[:, :n], 1.0,
                                           u, OP.add, OP.mult)
            rh01v = rh01.rearrange("p (t two w) -> p t two w", two=2, w=w)
            nc.tensor.matmul(pa[:, :n], wh[:, 0, :], rh01v[:, :ntt, 0, :],
                             start=True, stop=False)
            nc.tensor.matmul(pa[:, :n], wh[:, 1, :], rh01v[:, :ntt, 1, :],
                             start=False, stop=False)
            nc.tensor.matmul(pa[:, :n], wh[:, 2, :], rh2[:, :n],
                             start=False, stop=True)
            hH = hp2[:, half * MMT:half * MMT + n]
            nc.scalar.activation(hH, pa[:, :n], AF.Relu, bias=bv[:, 4:5], scale=cw)
            conv_tail(hH)
            # ---- z diffs d_m = z_m - z_3; hH accumulated LAST ----
            zk = [hH, hL, hR, u]
            e12pv = e12p.rearrange("p (t two w) -> p t two w", two=2, w=w)
            for m in range(3):
                for k in (1, 2, 3, 0):
                    nc.tensor.matmul(pbs[m][:, :n], wz[:, k, m, :], zk[k],
                                     start=(k == 1), stop=(k == 0))
            nc.scalar.activation(e0p[:, half * MMT:half * MMT + n], pbs[0][:, :n],
                                 AF.Exp, bias=bv[:, 5:6])
            nc.scalar.activation(e12pv[:, tr:tr + ntt, 0, :], pbs[1][:, :n],
                                 AF.Exp, bias=bv[:, 6:7])
            nc.scalar.activation(e12pv[:, tr:tr + ntt, 1, :], pbs[2][:, :n],
                                 AF.Exp, bias=bv[:, 7:8])

        def post_pair(cbuf, w, s0, n2, pair, out_ap):
            """Softmax-combine for a pair of tiles (n2 nodes) in one batch."""
            u2, hp2, e0p, e12p, _ = pair
            nt2 = n2 // w
            e12pv = e12p.rearrange("p (t two w) -> p t two w", two=2, w=w)
            cb_pair = cbuf[:, 2 * s0:2 * s0 + 2 * n2]
            # u2/hp2/e0p halves are [0:n) and [MMT:MMT+n): for full pairs this
            # is contiguous [0:2*MMT); singleton pairs use [0:n2) only.
            uflat = u2[:, :n2] if n2 <= MMT else u2[:, :2 * MMT]
            hflat = hp2[:, :n2] if n2 <= MMT else hp2[:, :2 * MMT]
            e0flat = e0p[:, :n2] if n2 <= MMT else e0p[:, :2 * MMT]
            # ---- denominator s = 1 + e0 + e1 + e2 ----
            s1 = spool.tile([H, 2 * MMT], bf16, name="s1", tag="s1", bufs=2)
            nc.gpsimd.tensor_tensor(s1[:, :n2], e0flat,
                                    e12pv[:, :nt2, 0, :], OP.add)
            sf = spool.tile([H, 2 * MMT], f32, name="sf", tag="sf", bufs=2)
            nc.vector.scalar_tensor_tensor(sf[:, :n2], s1[:, :n2], 1.0,
                                           e12pv[:, :nt2, 1, :], OP.add, OP.add)
            rcp = spool.tile([H, 2 * MMT], f32, name="rcp", tag="rcp", bufs=2)
            nc.vector.reciprocal_approx_fast(rcp[:, :n2], sf[:, :n2])
            # ---- numerator = e0*hH + e1*hL + e2*hR + u ----
            p12 = spool.tile([H, 4 * MMT], bf16, name="p12", tag="p12", bufs=2)
            nc.vector.tensor_tensor(p12[:, :2 * n2], e12p[:, :2 * n2],
                                    cb_pair, OP.mult)
            p0 = spool.tile([H, 2 * MMT], bf16, name="p0", tag="p0", bufs=2)
            nc.gpsimd.tensor_tensor(p0[:, :n2], e0flat, hflat, OP.mult)
            p12v = p12.rearrange("p (t two w) -> p t two w", two=2, w=w)
            bb = spool.tile([H, 2 * MMT], bf16, name="bb", tag="bb", bufs=2)
            bbv = bb.rearrange("p (t w) -> p t w", w=w)
            nc.gpsimd.tensor_tensor(bbv[:, :nt2, :], p12v[:, :nt2, 0, :],
                                    p12v[:, :nt2, 1, :], OP.add)
            aa = spool.tile([H, 2 * MMT], bf16, name="aa", tag="aa", bufs=2)
            nc.vector.tensor_tensor(aa[:, :n2], p0[:, :n2], uflat, OP.add)
            num = spool.tile([H, 2 * MMT], bf16, name="num", tag="num", bufs=2)
            nc.vector.tensor_tensor(num[:, :n2], aa[:, :n2], bb[:, :n2], OP.add)
            nc.vector.tensor_tensor(out_ap, num[:, :n2], rcp[:, :n2], OP.mult)

        def run_level(nj, w, ct_base, cbuf, obuf):
            """One level with nj parents of per-tree width w, in tile pairs."""
            done = 0
            while done < nj:
                piece = min(2048, nj - done)
                ctt = ctpool.tile([FEAT, 2048], bf16, name="ctt", tag="ctt")
                nc.sync.dma_start(ctt[:, :piece],
                                  ct_d[:, ct_base + done:ct_base + done + piece])
                for s in range(0, piece, 2 * MMT):
                    s0 = done + s
                    n2 = min(2 * MMT, piece - s)
                    u2 = spool.tile([H, 2 * MMT], bf16, name="u2", tag="u2")
                    hp2 = spool.tile([H, 2 * MMT], bf16, name="hp2", tag="hp2")
                    e0p = spool.tile([H, 2 * MMT], bf16, name="e0p", tag="e0p")
                    e12p = spool.tile([H, 4 * MMT], bf16, name="e12p", tag="e12p")
                    pair = (u2, hp2, e0p, e12p, s0)
                    for half in range(0, (n2 + MMT - 1) // MMT):
                        sb = s0 + half * MMT
                        n = min(MMT, n2 - half * MMT)
                        front_tile(cbuf, w, sb, n, ctt[:, s + half * MMT:
                                                       s + half * MMT + n],
                                   half, pair)
                    post_pair(cbuf, w, s0, n2, pair, obuf[:, s0:s0 + n2])
                done += piece

        # ================= phase A: per-chunk levels 9..5 =================
        for c in range(NCHUNK):
            # leaf level 9
            nleaf = TCH * 512  # 8192
            base9 = LOFF[9] + c * nleaf
            for hpiece in range(0, nleaf, 2048):
                ctt = ctpool.tile([FEAT, 2048], bf16, name="ctt", tag="ctt")
                nc.sync.dma_start(ctt[:], ct_d[:, base9 + hpiece:base9 + hpiece + 2048])
                for s in range(0, 2048, MMT):
                    p = pcnt[0] % 2
                    pcnt[0] += 1
                    pu = pp.tile([H, MMT], f32, name="pu", tag=f"pa{p}")
                    nc.tensor.matmul(pu[:], wu[:], ctt[:, s:s + MMT],
                                     start=True, stop=True)
                    dst = e9[:, hpiece + s:hpiece + s + MMT]
                    nc.scalar.activation(dst, pu[:], AF.Relu,
                                         bias=bv[:, 0:1], scale=cw)
                    if not collapsible:
                        nc.scalar.activation(dst, dst, AF.Relu, bias=cb, scale=cw)
                        nc.scalar.activation(dst, dst, AF.Relu, bias=cb, scale=cw)
                if collapsible and do_affine:
                    big = e9[:, hpiece:hpiece + 2048]
                    nc.vector.tensor_scalar(big, big, A, C, OP.mult, OP.add)
            # inner levels 8..5
            for j, (cbuf, obuf) in zip(
                    range(8, 4, -1),
                    [(e9, e8), (e8, e7), (e7, e6), (e6, None)]):
                nj = TCH * (2 ** j)
                if j == 5:
                    ob = emb5[:, c * 512:(c + 1) * 512]
                else:
                    ob = obuf[:, :nj]
                run_level(nj, 2 ** j, LOFF[j] + c * nj, cbuf[:], ob)

        # ================= phase B: levels 4..0, all trees =================
        # reuse dead phase-A buffers for the tail levels
        e4 = e8[:, :2048]
        e3 = e7[:, :1024]
        e2 = e6[:, :512]
        e1 = e8[:, 2048:2048 + 256]
        e0f = epool.tile([H, TPC], f32, tag="e0f")
        e0t = e0f[:, :TPC]
        chain = [(emb5[:], e4), (e4, e3), (e3, e2), (e2, e1), (e1, e0t)]
        for j, (cbap, ob) in zip(range(4, -1, -1), chain):
            nj = TPC * (2 ** j)
            run_level(nj, 2 ** j, LOFF[j], cbap, ob)

        # ================= output transpose + store =================
        pt = pp.tile([H, H], f32, name="pt", tag="pa0")
        nc.tensor.matmul(pt[:], e0t, idt[:], is_transpose=True, start=True, stop=True)
        osb = spool.tile([H, H], f32, name="osb", tag="osb")
        nc.vector.tensor_copy(osb[:], pt[:])
        nc.sync.dma_start(out_d[:], osb[:])

    nc.compile()
    if not nc.is_finalized():
        nc.finalize()
    return nc


def _prepare(inputs):
    contents = np.ascontiguousarray(np.asarray(inputs["contents"], np.float32))
    W_u = np.asarray(inputs["W_u"], np.float32)
    b_u = np.asarray(inputs["b_u"], np.float32)
    W_h = np.asarray(inputs["W_h"], np.float32)
    b_h = np.asarray(inputs["b_h"], np.float32)
    W_z = np.asarray(inputs["W_z"], np.float32)
    b_z = np.asarray(inputs["b_z"], np.float32)
    W_r = np.asarray(inputs["W_r"], np.float32)
    b_r = np.asarray(inputs["b_r"], np.float32)
    cw = float(np.asarray(inputs["conv_w"]).reshape(-1)[0])
    cb = float(np.asarray(inputs["conv_b"]).reshape(-1)[0])

    # per-core feature-major contents, level-major columns, bit-reversed
    # per-tree node order within each level
    cts = np.empty((NCORES, FEAT, NPC), np.float32)
    col = 0
    for j in range(L):
        n = TPC * 2 ** j
        blk = contents[OFF[j]:OFF[j + 1]].reshape(NCORES, TPC, 2 ** j, FEAT)
        if j > 0:
            blk = blk[:, :, _bitrev_perm(j), :]
        blk = blk.reshape(NCORES, n, FEAT)
        cts[:, :, col:col + n] = blk.transpose(0, 2, 1)
        col += n

    wr_np = np.ascontiguousarray(W_r.reshape(3, H, 3, H).transpose(1, 0, 2, 3))
    wh_np = np.ascontiguousarray((0.5 * W_h).reshape(3, H, H).transpose(1, 0, 2))
    # z-diff weights: Wd[k, :, m, :] = W_z[k,:,m,:] - W_z[k,:,3,:] for m=0..2
    wz4 = W_z.reshape(4, H, 4, H)
    wzd = np.ascontiguousarray(
        (wz4[:, :, 0:3, :] - wz4[:, :, 3:4, :]).transpose(1, 0, 2, 3))

    bvec = np.zeros((H, 8), np.float32)
    bvec[:, 0] = cw * b_u + cb
    bvec[:, 1:4] = 0.5 * b_r.reshape(3, H).T
    bvec[:, 4] = cw * b_h + cb
    bz4 = b_z.reshape(4, H)
    bvec[:, 5:8] = (bz4[0:3] - bz4[3:4]).T

    import ml_dtypes

    bf = ml_dtypes.bfloat16
    common = {
        "wu": np.ascontiguousarray(W_u).astype(bf),
        "wr": wr_np.astype(bf), "wh": wh_np.astype(bf), "wz": wzd.astype(bf),
        "bvec": bvec,
        "ident": np.eye(H, dtype=np.float32),
    }
    in_maps = [dict(common, ct=np.ascontiguousarray(cts[c]).astype(bf))
               for c in range(NCORES)]
    return in_maps, cw, cb


def kernel(**inputs):
    children = np.asarray(inputs["children"])
    cw = float(np.asarray(inputs["conv_w"]).reshape(-1)[0])
    cb = float(np.asarray(inputs["conv_b"]).reshape(-1)[0])
    collapsible = (cw >= 0.0) and (cb >= 0.0)
    if not _children_canonical(children):
        args = {k: np.asarray(v) for k, v in inputs.items()}
        return _numpy_fallback(**args)

    from concourse.bass_utils import run_bass_kernel_spmd

    A = cw * cw
    C = cw * cb + cb
    do_affine = not (A == 1.0 and C == 0.0)

    key = (cw, cb, collapsible, do_affine)
    if key not in _CACHE:
        _CACHE[key] = _build(cw, cb, collapsible, do_affine, A, C)
    nc = _CACHE[key]

    in_maps, _, _ = _prepare(inputs)
    res = run_bass_kernel_spmd(nc, in_maps, list(range(NCORES)))
    outs = [res.results[c]["out"] for c in range(NCORES)]
    return np.ascontiguousarray(np.concatenate(outs, axis=0).astype(np.float32))


if __name__ == "__main__":
    rng = np.random.default_rng(0)
    print("kernel module loaded")


# revision 9
# speedup vs baseline: 1.0290x; 1.0290x over previous
"""Trainium2 Bass kernel for nn_GRNNTransformGated (recursive tree GRNN over
1024 independent 10-level binary jets).

Strategy (v2):
  - Data-parallel over jets: 8 cores x 128 trees each.
  - BIT-REVERSED per-tree node layout: storing level j in bit-reversed local
    order makes the two children of parent position q land at positions q and
    q + 2^j of the level below -- every child "gather" becomes two contiguous
    stride-1 slices, so all DVE ops run in packed bf16 2x mode.
  - Softmax shift-invariance: z gates computed as differences d_m = z_m - z_3,
    so the z matmul is 4Hx3H (12 matmuls) instead of 4Hx4H (16), only 3 exps,
    e3 == 1 (numerator gets +u, denominator gets +1).
  - Feature-major layout [128 channels (partitions), nodes (free)]; weight
    blocks stationary, 25 matmuls per 512-node tile.
  - 8 separate PSUM banks (pu, pr0-2, ph, pz0-2) so consecutive tiles overlap
    with only early-read WAR dependencies -- keeps the PE dense and the HAM
    clock-gate warm (K=8/8).
  - conv_chain collapses: for w>0, b>=0, f(f(f(x))) = w^2*relu(w*x+b) + (w*b+b).
  - sigmoid via tanh; the 0.5 is folded into W_h on the host.
  - Per-tree interleaved temporaries (t01, e12, p12) let pairs of elementwise
    ops fuse into single FD=1024 stride-1 instructions.
"""

import sys

for _p in ("/opt/trn_rl_repo", "/root/.axon_site/_ro/trn_rl_repo"):
    if _p not in sys.path:
        sys.path.insert(0, _p)

import numpy as np

B = 1024
L = 10
H = 128
FEAT = 7
NCORES = 8
TPC = B // NCORES          # trees per core = 128
TCH = 16                   # trees per chunk
NCHUNK = TPC // TCH        # 8 chunks
NPC = TPC * (2 ** L - 1)   # nodes per core = 130944
LOFF = [TPC * (2 ** j - 1) for j in range(L + 1)]  # level offsets in per-core ct
LEVEL_SIZES = [B * 2 ** j for j in range(L)]
OFF = np.concatenate([[0], np.cumsum(LEVEL_SIZES)]).astype(int)
INNER = LEVEL_SIZES[:-1]
COFF = np.concatenate([[0], np.cumsum(INNER)]).astype(int)

MMT = 512  # matmul node-tile size

_CACHE = {}


def _children_canonical(children):
    for j in range(L - 1):
        n = INNER[j]
        blk = children[COFF[j]:COFF[j + 1]]
        base = 2 * np.arange(n, dtype=np.int64)
        if not (np.array_equal(blk[:, 0], base) and np.array_equal(blk[:, 1], base + 1)):
            return False
    return True


def _numpy_fallback(contents, children, W_u, b_u, W_h, b_h, W_z, b_z, W_r, b_r,
                    conv_w, conv_b):
    w, b = float(conv_w[0]), float(conv_b[0])

    def conv_chain(x):
        for _ in range(3):
            x = np.maximum(w * x + b, 0.0)
        return x

    def sigmoid(x):
        return 1.0 / (1.0 + np.exp(-x))

    emb = None
    for j in reversed(range(L)):
        c = contents[OFF[j]:OFF[j + 1]]
        u = conv_chain(c @ W_u + b_u)
        if j == L - 1:
            emb = u
            continue
        ch = children[COFF[j]:COFF[j + 1]]
        h_L = emb[ch[:, 0]]
        h_R = emb[ch[:, 1]]
        hhu = np.concatenate([h_L, h_R, u], axis=1)
        r = sigmoid(hhu @ W_r + b_r)
        h_H = conv_chain((r * hhu) @ W_h + b_h)
        z = np.concatenate([h_H, hhu], axis=1) @ W_z + b_z
        zs = np.stack([z[:, :H], z[:, H:2 * H], z[:, 2 * H:3 * H], z[:, 3 * H:]], axis=-1)
        zs = zs - zs.max(axis=-1, keepdims=True)
        e = np.exp(zs)
        g = e / e.sum(axis=-1, keepdims=True)
        emb = g[..., 0] * h_H + g[..., 1] * h_L + g[..., 2] * h_R + g[..., 3] * u
    return emb.reshape(B, -1).astype(np.float32)


def _bitrev_perm(j):
    """perm[q] = bit-reverse of q over j bits."""
    if j == 0:
        return np.zeros(1, dtype=np.int64)
    return (
        np.arange(2 ** j, dtype=np.int64)
        .reshape((2,) * j)
        .transpose(tuple(reversed(range(j))))
        .ravel()
    )


def _build(cw, cb, collapsible, do_affine, A, C):
    from contextlib import ExitStack

    from concourse import bacc, bass, mybir, tile

    f32 = mybir.dt.float32
    bf16 = mybir.dt.bfloat16
    AF = mybir.ActivationFunctionType
    OP = mybir.AluOpType

    nc = bacc.Bacc()

    ct_d = nc.declare_dram_parameter("ct", [FEAT, NPC], bf16, isOutput=False)
    wu_d = nc.declare_dram_parameter("wu", [FEAT, H], bf16, isOutput=False)
    wr_d = nc.declare_dram_parameter("wr", [H, 3, 3, H], bf16, isOutput=False)
    wh_d = nc.declare_dram_parameter("wh", [H, 3, H], bf16, isOutput=False)
    wz_d = nc.declare_dram_parameter("wz", [H, 4, 3, H], bf16, isOutput=False)
    bv_d = nc.declare_dram_parameter("bvec", [H, 8], f32, isOutput=False)
    id_d = nc.declare_dram_parameter("ident", [H, H], f32, isOutput=False)
    out_d = nc.declare_dram_parameter("out", [TPC, H], f32, isOutput=True)

    with ExitStack() as ctx:
        tc = ctx.enter_context(tile.TileContext(nc))
        wpool = ctx.enter_context(tc.tile_pool(name="wts", bufs=1))
        epool = ctx.enter_context(tc.tile_pool(name="emb", bufs=1))
        ctpool = ctx.enter_context(tc.tile_pool(name="ct", bufs=3))
        spool = ctx.enter_context(tc.tile_pool(name="tmp", bufs=3))
        pp = ctx.enter_context(tc.tile_pool(name="ps", bufs=1, space="PSUM"))

        wu = wpool.tile([FEAT, H], bf16, tag="wu")
        wr = wpool.tile([H, 3, 3, H], bf16, tag="wr")
        wh = wpool.tile([H, 3, H], bf16, tag="wh")
        wz = wpool.tile([H, 4, 3, H], bf16, tag="wz")
        bv = wpool.tile([H, 8], f32, tag="bv")
        idt = wpool.tile([H, H], f32, tag="idt")
        nc.sync.dma_start(wu[:], wu_d[:])
        nc.sync.dma_start(wr[:], wr_d[:])
        nc.sync.dma_start(wh[:], wh_d[:])
        nc.sync.dma_start(wz[:], wz_d[:])
        nc.sync.dma_start(bv[:], bv_d[:])
        nc.sync.dma_start(idt[:], id_d[:])

        # emb level buffers (phase A holds one chunk; emb5 accumulates all chunks)
        e9 = epool.tile([H, TCH * 512], bf16, tag="e9")     # 8192
        e8 = epool.tile([H, TCH * 256], bf16, tag="e8")     # 4096
        e7 = epool.tile([H, TCH * 128], bf16, tag="e7")     # 2048
        e6 = epool.tile([H, TCH * 64], bf16, tag="e6")      # 1024
        emb5 = epool.tile([H, TPC * 32], bf16, tag="emb5")  # 4096 (all trees)

        zeros = wpool.tile([H, 2 * MMT], bf16, tag="zeros")
        nc.vector.memset(zeros[:], 0.0)

        pcnt = [0]  # global front-tile parity counter

        def conv_tail(dst):
            if collapsible:
                if do_affine:
                    nc.vector.tensor_scalar(dst, dst, A, C, OP.mult, OP.add)
            else:
                nc.scalar.activation(dst, dst, AF.Relu, bias=cb, scale=cw)
                nc.scalar.activation(dst, dst, AF.Relu, bias=cb, scale=cw)

        class FrontTile:
            """Matmul/gate front-end for one 512-node tile, split into stages
            so that two tiles of a pair can be emitted stage-interleaved
            (keeps every engine queue in data-ready order)."""

            def __init__(self, cbuf, w, s, n, ct_ap, half, pair):
                self.p = pcnt[0] % 2
                pcnt[0] += 1
                self.w, self.s, self.n, self.ct_ap, self.half = w, s, n, ct_ap, half
                self.u2, self.hp2, self.e0p, self.e12p, s0 = pair
                self.ntt = n // w
                t0 = s // w
                self.tr = (s - s0) // w                     # tree offset in pair
                cb4 = cbuf.rearrange("p (t two w) -> p t two w", two=2, w=w)
                self.hL = cb4[:, t0:t0 + self.ntt, 0, :]    # [H, ntt, w]
                self.hR = cb4[:, t0:t0 + self.ntt, 1, :]
                self.cb_both = cbuf[:, 2 * s:2 * s + 2 * n]

            def stage_u(self):
                n, p = self.n, self.p
                self.pa = pp.tile([H, MMT], f32, name="pa", tag=f"pa{p}")
                nc.tensor.matmul(self.pa[:, :n], wu[:], self.ct_ap,
                                 start=True, stop=True)
                self.u = self.u2[:, self.half * MMT:self.half * MMT + n]
                nc.scalar.activation(self.u, self.pa[:, :n], AF.Relu,
                                     bias=bv[:, 0:1], scale=cw)
                conv_tail(self.u)

            def stage_r(self):
                n, p, w, ntt = self.n, self.p, self.w, self.ntt
                self.pbs = [pp.tile([H, MMT], f32, name=f"pb{m}", tag=f"pb{m}_{p}")
                            for m in range(3)]
                rhs_k = [self.hL, self.hR, self.u]
                for m in range(3):
                    for k in range(3):
                        nc.tensor.matmul(self.pbs[m][:, :n], wr[:, k, m, :],
                                         rhs_k[k], start=(k == 0), stop=(k == 2))
                self.t01 = spool.tile([H, 2 * MMT], bf16, name="t01", tag="t01",
                                      bufs=2)
                t01v = self.t01.rearrange("p (t two w) -> p t two w", two=2, w=w)
                self.t2 = spool.tile([H, MMT], bf16, name="t2", tag="t2", bufs=2)
                for m in range(2):
                    nc.scalar.activation(t01v[:, :ntt, m, :], self.pbs[m][:, :n],
                                         AF.Tanh, bias=bv[:, 1 + m:2 + m],
                                         scale=0.5)
                nc.scalar.activation(self.t2[:, :n], self.pbs[2][:, :n], AF.Tanh,
                                     bias=bv[:, 3:4], scale=0.5)

            def stage_h(self):
                n, w, ntt = self.n, self.w, self.ntt
                rh01 = spool.tile([H, 2 * MMT], bf16, name="rh01", tag="rh01",
                                  bufs=2)
                nc.vector.scalar_tensor_tensor(rh01[:, :2 * n], self.t01[:, :2 * n],
                                               1.0, self.cb_both, OP.add, OP.mult)
                rh2 = spool.tile([H, MMT], bf16, name="rh2", tag="rh2", bufs=2)
                nc.vector.scalar_tensor_tensor(rh2[:, :n], self.t2[:, :n], 1.0,
                                               self.u, OP.add, OP.mult)
                rh01v = rh01.rearrange("p (t two w) -> p t two w", two=2, w=w)
                nc.tensor.matmul(self.pa[:, :n], wh[:, 0, :], rh01v[:, :ntt, 0, :],
                                 start=True, stop=False)
                nc.tensor.matmul(self.pa[:, :n], wh[:, 1, :], rh01v[:, :ntt, 1, :],
                                 start=False, stop=False)
                nc.tensor.matmul(self.pa[:, :n], wh[:, 2, :], rh2[:, :n],
                                 start=False, stop=True)
                self.hH = self.hp2[:, self.half * MMT:self.half * MMT + n]
                nc.scalar.activation(self.hH, self.pa[:, :n], AF.Relu,
                                     bias=bv[:, 4:5], scale=cw)
                conv_tail(self.hH)

            def stage_z(self):
                n, w, ntt, tr = self.n, self.w, self.ntt, self.tr
                zk = [self.hH, self.hL, self.hR, self.u]
                e12pv = self.e12p.rearrange("p (t two w) -> p t two w", two=2, w=w)
                for m in range(3):
                    for k in (1, 2, 3, 0):
                        nc.tensor.matmul(self.pbs[m][:, :n], wz[:, k, m, :],
                                         zk[k], start=(k == 1), stop=(k == 0))
                nc.scalar.activation(self.e0p[:, self.half * MMT:
                                              self.half * MMT + n],
                                     self.pbs[0][:, :n], AF.Exp, bias=bv[:, 5:6])
                nc.scalar.activation(e12pv[:, tr:tr + ntt, 0, :],
                                     self.pbs[1][:, :n], AF.Exp, bias=bv[:, 6:7])
                nc.scalar.activation(e12pv[:, tr:tr + ntt, 1, :],
                                     self.pbs[2][:, :n], AF.Exp, bias=bv[:, 7:8])

        def post_pair(cbuf, w, s0, n2, pair, out_ap):
            """Softmax-combine for a pair of tiles (n2 nodes) in one batch."""
            u2, hp2, e0p, e12p, _ = pair
            nt2 = n2 // w
            e12pv = e12p.rearrange("p (t two w) -> p t two w", two=2, w=w)
            cb_pair = cbuf[:, 2 * s0:2 * s0 + 2 * n2]
            # u2/hp2/e0p halves are [0:n) and [MMT:MMT+n): for full pairs this
            # is contiguous [0:2*MMT); singleton pairs use [0:n2) only.
            uflat = u2[:, :n2] if n2 <= MMT else u2[:, :2 * MMT]
            hflat = hp2[:, :n2] if n2 <= MMT else hp2[:, :2 * MMT]
            e0flat = e0p[:, :n2] if n2 <= MMT else e0p[:, :2 * MMT]
            # ---- denominator s = 1 + e0 + e1 + e2 ----
            s1 = spool.tile([H, 2 * MMT], bf16, name="s1", tag="s1", bufs=2)
            nc.gpsimd.tensor_tensor(s1[:, :n2], e0flat,
                                    e12pv[:, :nt2, 0, :], OP.add)
            sf = spool.tile([H, 2 * MMT], f32, name="sf", tag="sf", bufs=2)
            nc.vector.scalar_tensor_tensor(sf[:, :n2], s1[:, :n2], 1.0,
                                           e12pv[:, :nt2, 1, :], OP.add, OP.add)
            rcp = spool.tile([H, 2 * MMT], f32, name="rcp", tag="rcp", bufs=2)
            nc.vector.reciprocal_approx_fast(rcp[:, :n2], sf[:, :n2])
            # ---- numerator = e0*hH + e1*hL + e2*hR + u ----
            p12 = spool.tile([H, 4 * MMT], bf16, name="p12", tag="p12", bufs=2)
            nc.vector.tensor_tensor(p12[:, :2 * n2], e12p[:, :2 * n2],
                                    cb_pair, OP.mult)
            p0 = spool.tile([H, 2 * MMT], bf16, name="p0", tag="p0", bufs=2)
            nc.gpsimd.tensor_tensor(p0[:, :n2], e0flat, hflat, OP.mult)
            p12v = p12.rearrange("p (t two w) -> p t two w", two=2, w=w)
            bb = spool.tile([H, 2 * MMT], bf16, name="bb", tag="bb", bufs=2)
            bbv = bb.rearrange("p (t w) -> p t w", w=w)
            nc.gpsimd.tensor_tensor(bbv[:, :nt2, :], p12v[:, :nt2, 0, :],
                                    p12v[:, :nt2, 1, :], OP.add)
            aa = spool.tile([H, 2 * MMT], bf16, name="aa", tag="aa", bufs=2)
            nc.vector.tensor_tensor(aa[:, :n2], p0[:, :n2], uflat, OP.add)
            num = spool.tile([H, 2 * MMT], bf16, name="num", tag="num", bufs=2)
            nc.vector.tensor_tensor(num[:, :n2], aa[:, :n2], bb[:, :n2], OP.add)
            nc.vector.tensor_tensor(out_ap, num[:, :n2], rcp[:, :n2], OP.mult)

        def run_level(nj, w, ct_base, cbuf, obuf):
            """One level with nj parents of per-tree width w, in tile pairs."""
            done = 0
            while done < nj:
                piece = min(2048, nj - done)
                ctt = ctpool.tile([FEAT, 2048], bf16, name="ctt", tag="ctt")
                nc.sync.dma_start(ctt[:, :piece],
                                  ct_d[:, ct_base + done:ct_base + done + piece])
                for s in range(0, piece, 2 * MMT):
                    s0 = done + s
                    n2 = min(2 * MMT, piece - s)
                    u2 = spool.tile([H, 2 * MMT], bf16, name="u2", tag="u2")
                    hp2 = spool.tile([H, 2 * MMT], bf16, name="hp2", tag="hp2")
                    e0p = spool.tile([H, 2 * MMT], bf16, name="e0p", tag="e0p")
                    e12p = spool.tile([H, 4 * MMT], bf16, name="e12p", tag="e12p")
                    pair = (u2, hp2, e0p, e12p, s0)
                    fts = []
                    for half in range(0, (n2 + MMT - 1) // MMT):
                        sb = s0 + half * MMT
                        n = min(MMT, n2 - half * MMT)
                        fts.append(FrontTile(cbuf, w, sb, n,
                                             ctt[:, s + half * MMT:
                                                 s + half * MMT + n],
                                             half, pair))
                    for stage in ("stage_u", "stage_r", "stage_h", "stage_z"):
                        for ft in fts:
                            getattr(ft, stage)()
                    post_pair(cbuf, w, s0, n2, pair, obuf[:, s0:s0 + n2])
                done += piece

        # ================= phase A: per-chunk levels 9..5 =================
        for c in range(NCHUNK):
            # leaf level 9
            nleaf = TCH * 512  # 8192
            base9 = LOFF[9] + c * nleaf
            for hpiece in range(0, nleaf, 2048):
                ctt = ctpool.tile([FEAT, 2048], bf16, name="ctt", tag="ctt")
                nc.sync.dma_start(ctt[:], ct_d[:, base9 + hpiece:base9 + hpiece + 2048])
                for s in range(0, 2048, MMT):
                    p = pcnt[0] % 2
                    pcnt[0] += 1
                    pu = pp.tile([H, MMT], f32, name="pu", tag=f"pa{p}")
                    nc.tensor.matmul(pu[:], wu[:], ctt[:, s:s + MMT],
                                     start=True, stop=True)
                    dst = e9[:, hpiece + s:hpiece + s + MMT]
                    nc.scalar.activation(dst, pu[:], AF.Relu,
                                         bias=bv[:, 0:1], scale=cw)
                    if not collapsible:
                        nc.scalar.activation(dst, dst, AF.Relu, bias=cb, scale=cw)
                        nc.scalar.activation(dst, dst, AF.Relu, bias=cb, scale=cw)
                if collapsible and do_affine:
                    big = e9[:, hpiece:hpiece + 2048]
                    nc.vector.tensor_scalar(big, big, A, C, OP.mult, OP.add)
            # inner levels 8..5
            for j, (cbuf, obuf) in zip(
                    range(8, 4, -1),
                    [(e9, e8), (e8, e7), (e7, e6), (e6, None)]):
                nj = TCH * (2 ** j)
                if j == 5:
                    ob = emb5[:, c * 512:(c + 1) * 512]
                else:
                    ob = obuf[:, :nj]
                run_level(nj, 2 ** j, LOFF[j] + c * nj, cbuf[:], ob)

        # ================= phase B: levels 4..0, all trees =================
        # reuse dead phase-A buffers for the tail levels
        e4 = e8[:, :2048]
        e3 = e7[:, :1024]
        e2 = e6[:, :512]
        e1 = e8[:, 2048:2048 + 256]
        e0f = epool.tile([H, TPC], f32, tag="e0f")
        e0t = e0f[:, :TPC]
        chain = [(emb5[:], e4), (e4, e3), (e3, e2), (e2, e1), (e1, e0t)]
        for j, (cbap, ob) in zip(range(4, -1, -1), chain):
            nj = TPC * (2 ** j)
            run_level(nj, 2 ** j, LOFF[j], cbap, ob)

        # ================= output transpose + store =================
        pt = pp.tile([H, H], f32, name="pt", tag="pa0")
        nc.tensor.matmul(pt[:], e0t, idt[:], is_transpose=True, start=True, stop=True)
        osb = spool.tile([H, H], f32, name="osb", tag="osb")
        nc.vector.tensor_copy(osb[:], pt[:])
        nc.sync.dma_start(out_d[:], osb[:])

    nc.compile()
    if not nc.is_finalized():
        nc.finalize()
    return nc


def _prepare(inputs):
    contents = np.ascontiguousarray(np.asarray(inputs["contents"], np.float32))
    W_u = np.asarray(inputs["W_u"], np.float32)
    b_u = np.asarray(inputs["b_u"], np.float32)
    W_h = np.asarray(inputs["W_h"], np.float32)
    b_h = np.asarray(inputs["b_h"], np.float32)
    W_z = np.asarray(inputs["W_z"], np.float32)
    b_z = np.asarray(inputs["b_z"], np.float32)
    W_r = np.asarray(inputs["W_r"], np.float32)
    b_r = np.asarray(inputs["b_r"], np.float32)
    cw = float(np.asarray(inputs["conv_w"]).reshape(-1)[0])
    cb = float(np.asarray(inputs["conv_b"]).reshape(-1)[0])

    # per-core feature-major contents, level-major columns, bit-reversed
    # per-tree node order within each level
    cts = np.empty((NCORES, FEAT, NPC), np.float32)
    col = 0
    for j in range(L):
        n = TPC * 2 ** j
        blk = contents[OFF[j]:OFF[j + 1]].reshape(NCORES, TPC, 2 ** j, FEAT)
        if j > 0:
            blk = blk[:, :, _bitrev_perm(j), :]
        blk = blk.reshape(NCORES, n, FEAT)
        cts[:, :, col:col + n] = blk.transpose(0, 2, 1)
        col += n

    wr_np = np.ascontiguousarray(W_r.reshape(3, H, 3, H).transpose(1, 0, 2, 3))
    wh_np = np.ascontiguousarray((0.5 * W_h).reshape(3, H, H).transpose(1, 0, 2))
    # z-diff weights: Wd[k, :, m, :] = W_z[k,:,m,:] - W_z[k,:,3,:] for m=0..2
    wz4 = W_z.reshape(4, H, 4, H)
    wzd = np.ascontiguousarray(
        (wz4[:, :, 0:3, :] - wz4[:, :, 3:4, :]).transpose(1, 0, 2, 3))

    bvec = np.zeros((H, 8), np.float32)
    bvec[:, 0] = cw * b_u + cb
    bvec[:, 1:4] = 0.5 * b_r.reshape(3, H).T
    bvec[:, 4] = cw * b_h + cb
    bz4 = b_z.reshape(4, H)
    bvec[:, 5:8] = (bz4[0:3] - bz4[3:4]).T

    import ml_dtypes

    bf = ml_dtypes.bfloat16
    common = {
        "wu": np.ascontiguousarray(W_u).astype(bf),
        "wr": wr_np.astype(bf), "wh": wh_np.astype(bf), "wz": wzd.astype(bf),
        "bvec": bvec,
        "ident": np.eye(H, dtype=np.float32),
    }
    in_maps = [dict(common, ct=np.ascontiguousarray(cts[c]).astype(bf))
               for c in range(NCORES)]
    return in_maps, cw, cb


def kernel(**inputs):
    children = np.asarray(inputs["children"])
    cw = float(np.asarray(inputs["conv_w"]).reshape(-1)[0])
    cb = float(np.asarray(inputs["conv_b"]).reshape(-1)[0])
    collapsible = (cw >= 0.0) and (cb >= 0.0)
    if not _children_canonical(children):
        args = {k: np.asarray(v) for k, v in inputs.items()}
        return _numpy_fallback(**args)

    from concourse.bass_utils import run_bass_kernel_spmd

    A = cw * cw
    C = cw * cb + cb
    do_affine = not (A == 1.0 and C == 0.0)

    key = (cw, cb, collapsible, do_affine)
    if key not in _CACHE:
        _CACHE[key] = _build(cw, cb, collapsible, do_affine, A, C)
    nc = _CACHE[key]

    in_maps, _, _ = _prepare(inputs)
    res = run_bass_kernel_spmd(nc, in_maps, list(range(NCORES)))
    outs = [res.results[c]["out"] for c in range(NCORES)]
    return np.ascontiguousarray(np.concatenate(outs, axis=0).astype(np.float32))


if __name__ == "__main__":
    rng = np.random.default_rng(0)
    print("kernel module loaded")


# revision 11
# speedup vs baseline: 1.3810x; 1.3421x over previous
"""Trainium2 Bass kernel for nn_GRNNTransformGated (recursive tree GRNN over
1024 independent 10-level binary jets).

Strategy (v2):
  - Data-parallel over jets: 8 cores x 128 trees each.
  - BIT-REVERSED per-tree node layout: storing level j in bit-reversed local
    order makes the two children of parent position q land at positions q and
    q + 2^j of the level below -- every child "gather" becomes two contiguous
    stride-1 slices, so all DVE ops run in packed bf16 2x mode.
  - Softmax shift-invariance: z gates computed as differences d_m = z_m - z_3,
    so the z matmul is 4Hx3H (12 matmuls) instead of 4Hx4H (16), only 3 exps,
    e3 == 1 (numerator gets +u, denominator gets +1).
  - Feature-major layout [128 channels (partitions), nodes (free)]; weight
    blocks stationary, 25 matmuls per 512-node tile.
  - 8 separate PSUM banks (pu, pr0-2, ph, pz0-2) so consecutive tiles overlap
    with only early-read WAR dependencies -- keeps the PE dense and the HAM
    clock-gate warm (K=8/8).
  - conv_chain collapses: for w>0, b>=0, f(f(f(x))) = w^2*relu(w*x+b) + (w*b+b).
  - sigmoid via tanh; the 0.5 is folded into W_h on the host.
  - Per-tree interleaved temporaries (t01, e12, p12) let pairs of elementwise
    ops fuse into single FD=1024 stride-1 instructions.
"""

import sys

for _p in ("/opt/trn_rl_repo", "/root/.axon_site/_ro/trn_rl_repo"):
    if _p not in sys.path:
        sys.path.insert(0, _p)

import numpy as np

B = 1024
L = 10
H = 128
FEAT = 7
NCORES = 8
TPC = B // NCORES          # trees per core = 128
TCH = 16                   # trees per chunk
NCHUNK = TPC // TCH        # 8 chunks
NPC = TPC * (2 ** L - 1)   # nodes per core = 130944
LOFF = [TPC * (2 ** j - 1) for j in range(L + 1)]  # level offsets in per-core ct
LEVEL_SIZES = [B * 2 ** j for j in range(L)]
OFF = np.concatenate([[0], np.cumsum(LEVEL_SIZES)]).astype(int)
INNER = LEVEL_SIZES[:-1]
COFF = np.concatenate([[0], np.cumsum(INNER)]).astype(int)

MMT = 512  # matmul node-tile size

_CACHE = {}


def _children_canonical(children):
    for j in range(L - 1):
        n = INNER[j]
        blk = children[COFF[j]:COFF[j + 1]]
        base = 2 * np.arange(n, dtype=np.int64)
        if not (np.array_equal(blk[:, 0], base) and np.array_equal(blk[:, 1], base + 1)):
            return False
    return True


def _numpy_fallback(contents, children, W_u, b_u, W_h, b_h, W_z, b_z, W_r, b_r,
                    conv_w, conv_b):
    w, b = float(conv_w[0]), float(conv_b[0])

    def conv_chain(x):
        for _ in range(3):
            x = np.maximum(w * x + b, 0.0)
        return x

    def sigmoid(x):
        return 1.0 / (1.0 + np.exp(-x))

    emb = None
    for j in reversed(range(L)):
        c = contents[OFF[j]:OFF[j + 1]]
        u = conv_chain(c @ W_u + b_u)
        if j == L - 1:
            emb = u
            continue
        ch = children[COFF[j]:COFF[j + 1]]
        h_L = emb[ch[:, 0]]
        h_R = emb[ch[:, 1]]
        hhu = np.concatenate([h_L, h_R, u], axis=1)
        r = sigmoid(hhu @ W_r + b_r)
        h_H = conv_chain((r * hhu) @ W_h + b_h)
        z = np.concatenate([h_H, hhu], axis=1) @ W_z + b_z
        zs = np.stack([z[:, :H], z[:, H:2 * H], z[:, 2 * H:3 * H], z[:, 3 * H:]], axis=-1)
        zs = zs - zs.max(axis=-1, keepdims=True)
        e = np.exp(zs)
        g = e / e.sum(axis=-1, keepdims=True)
        emb = g[..., 0] * h_H + g[..., 1] * h_L + g[..., 2] * h_R + g[..., 3] * u
    return emb.reshape(B, -1).astype(np.float32)


def _bitrev_perm(j):
    """perm[q] = bit-reverse of q over j bits."""
    if j == 0:
        return np.zeros(1, dtype=np.int64)
    return (
        np.arange(2 ** j, dtype=np.int64)
        .reshape((2,) * j)
        .transpose(tuple(reversed(range(j))))
        .ravel()
    )


def _build(cw, cb, collapsible, do_affine, A, C):
    from contextlib import ExitStack

    from concourse import bacc, bass, mybir, tile

    f32 = mybir.dt.float32
    bf16 = mybir.dt.bfloat16
    AF = mybir.ActivationFunctionType
    OP = mybir.AluOpType

    nc = bacc.Bacc()

    ct_d = nc.declare_dram_parameter("ct", [FEAT, NPC], bf16, isOutput=False)
    wu_d = nc.declare_dram_parameter("wu", [FEAT, H], bf16, isOutput=False)
    wr_d = nc.declare_dram_parameter("wr", [H, 3, 3, H], bf16, isOutput=False)
    wh_d = nc.declare_dram_parameter("wh", [H, 3, H], bf16, isOutput=False)
    wz_d = nc.declare_dram_parameter("wz", [H, 4, 3, H], bf16, isOutput=False)
    bv_d = nc.declare_dram_parameter("bvec", [H, 8], f32, isOutput=False)
    id_d = nc.declare_dram_parameter("ident", [H, H], f32, isOutput=False)
    out_d = nc.declare_dram_parameter("out", [TPC, H], f32, isOutput=True)

    with ExitStack() as ctx:
        tc = ctx.enter_context(tile.TileContext(nc))
        wpool = ctx.enter_context(tc.tile_pool(name="wts", bufs=1))
        epool = ctx.enter_context(tc.tile_pool(name="emb", bufs=1))
        ctpool = ctx.enter_context(tc.tile_pool(name="ct", bufs=3))
        spool = ctx.enter_context(tc.tile_pool(name="tmp", bufs=3))
        pp = ctx.enter_context(tc.tile_pool(name="ps", bufs=1, space="PSUM"))

        wu = wpool.tile([FEAT, H], bf16, tag="wu")
        wr = wpool.tile([H, 3, 3, H], bf16, tag="wr")
        wh = wpool.tile([H, 3, H], bf16, tag="wh")
        wz = wpool.tile([H, 4, 3, H], bf16, tag="wz")
        bv = wpool.tile([H, 8], f32, tag="bv")
        idt = wpool.tile([H, H], f32, tag="idt")
        nc.sync.dma_start(wu[:], wu_d[:])
        nc.sync.dma_start(wr[:], wr_d[:])
        nc.sync.dma_start(wh[:], wh_d[:])
        nc.sync.dma_start(wz[:], wz_d[:])
        nc.sync.dma_start(bv[:], bv_d[:])
        nc.sync.dma_start(idt[:], id_d[:])

        # emb level buffers (phase A holds one chunk; emb5 accumulates all chunks)
        e9 = epool.tile([H, TCH * 512], bf16, tag="e9")     # 8192
        e8 = epool.tile([H, TCH * 256], bf16, tag="e8")     # 4096
        e7 = epool.tile([H, TCH * 128], bf16, tag="e7")     # 2048
        e6 = epool.tile([H, TCH * 64], bf16, tag="e6")      # 1024
        emb5 = epool.tile([H, TPC * 32], bf16, tag="emb5")  # 4096 (all trees)

        zeros = wpool.tile([H, 2 * MMT], bf16, tag="zeros")
        nc.vector.memset(zeros[:], 0.0)

        def conv_tail(dst):
            if collapsible:
                if do_affine:
                    nc.vector.tensor_scalar(dst, dst, A, C, OP.mult, OP.add)
            else:
                nc.scalar.activation(dst, dst, AF.Relu, bias=cb, scale=cw)
                nc.scalar.activation(dst, dst, AF.Relu, bias=cb, scale=cw)

        class PairBlock:
            """One 1024-node tile pair: matmuls run per 512 half (PSUM bank
            limit), activations and DVE ops run pair-wide (FD=1024/2048).
            PSUM: pa (2 banks, u only), pb0..pb2 (2 banks each; pb0 also holds
            the h and z0 accumulations -- their WARs are implied by true deps).
            Post-softmax work is split into chunks that the driver interleaves
            with the NEXT pair's front stages, so every engine queue stays in
            data-ready order."""

            def __init__(self, cbuf, w, s0, n2, ct_ap, out_ap):
                self.w, self.s0, self.n2 = w, s0, n2
                self.ct_ap = ct_ap
                self.out_ap = out_ap
                self.nt2 = n2 // w
                self.t0 = s0 // w
                self.cb4 = cbuf.rearrange("p (t two w) -> p t two w", two=2, w=w)
                self.cb_pair = cbuf[:, 2 * s0:2 * s0 + 2 * n2]
                self.halves = []
                off = 0
                while off < n2:
                    n = min(MMT, n2 - off)
                    self.halves.append((off, n))
                    off += n

            def hLR(self, off, n):
                tt0 = self.t0 + off // self.w
                ntt = n // self.w
                return (self.cb4[:, tt0:tt0 + ntt, 0, :],
                        self.cb4[:, tt0:tt0 + ntt, 1, :])

            def stage_u(self):
                n2 = self.n2
                self.pa = pp.tile([H, 2 * MMT], f32, name="pa", tag="pa")
                for off, n in self.halves:
                    nc.tensor.matmul(self.pa[:, off:off + n], wu[:],
                                     self.ct_ap[:, off:off + n],
                                     start=True, stop=True)
                self.u2 = spool.tile([H, 2 * MMT], bf16, name="u2", tag="u2")
                self.u = self.u2[:, :n2]
                nc.scalar.activation(self.u, self.pa[:, :n2], AF.Relu,
                                     bias=bv[:, 0:1], scale=cw)
                conv_tail(self.u)

            def stage_r(self):
                n2, w, nt2 = self.n2, self.w, self.nt2
                self.pbs = [pp.tile([H, 2 * MMT], f32, name=f"pb{m}", tag=f"pb{m}")
                            for m in range(3)]
                for m in range(3):
                    for off, n in self.halves:
                        hL, hR = self.hLR(off, n)
                        rhs = [hL, hR, self.u2[:, off:off + n]]
                        for k in range(3):
                            nc.tensor.matmul(self.pbs[m][:, off:off + n],
                                             wr[:, k, m, :], rhs[k],
                                             start=(k == 0), stop=(k == 2))
                self.t01 = spool.tile([H, 4 * MMT], bf16, name="t01", tag="t01",
                                      bufs=2)
                t01v = self.t01.rearrange("p (t two w) -> p t two w", two=2, w=w)
                self.t2 = spool.tile([H, 2 * MMT], bf16, name="t2", tag="t2",
                                     bufs=2)
                for m in range(2):
                    nc.scalar.activation(t01v[:, :nt2, m, :], self.pbs[m][:, :n2],
                                         AF.Tanh, bias=bv[:, 1 + m:2 + m],
                                         scale=0.5)
                nc.scalar.activation(self.t2[:, :n2], self.pbs[2][:, :n2],
                                     AF.Tanh, bias=bv[:, 3:4], scale=0.5)

            def stage_h(self):
                # W_h-folded: h_psum = sum_k wh[k]@(t_k*x_k) + wh[k]@x_k
                # (the GRU 0.5*(t+1) is the half-sum of those two terms; the
                # 0.5 is pre-folded into wh on the host)
                n2, w = self.n2, self.w
                th01 = spool.tile([H, 4 * MMT], bf16, name="th01", tag="th01",
                                  bufs=2)
                nc.vector.tensor_tensor(th01[:, :2 * n2], self.t01[:, :2 * n2],
                                        self.cb_pair, OP.mult)
                th2 = spool.tile([H, 2 * MMT], bf16, name="th2", tag="th2",
                                 bufs=2)
                nc.vector.tensor_tensor(th2[:, :n2], self.t2[:, :n2], self.u,
                                        OP.mult)
                th01v = th01.rearrange("p (t two w) -> p t two w", two=2, w=w)
                ph = self.pbs[0]
                for off, n in self.halves:
                    tt0 = off // w
                    ntt = n // w
                    hL, hR = self.hLR(off, n)
                    terms = [
                        (wh[:, 0, :], th01v[:, tt0:tt0 + ntt, 0, :]),
                        (wh[:, 1, :], th01v[:, tt0:tt0 + ntt, 1, :]),
                        (wh[:, 2, :], th2[:, off:off + n]),
                        (wh[:, 0, :], hL),
                        (wh[:, 1, :], hR),
                        (wh[:, 2, :], self.u2[:, off:off + n]),
                    ]
                    for i, (wt, rhs) in enumerate(terms):
                        nc.tensor.matmul(ph[:, off:off + n], wt, rhs,
                                         start=(i == 0),
                                         stop=(i == len(terms) - 1))
                self.hp2 = spool.tile([H, 2 * MMT], bf16, name="hp2", tag="hp2")
                self.hH = self.hp2[:, :n2]
                nc.scalar.activation(self.hH, ph[:, :n2], AF.Relu,
                                     bias=bv[:, 4:5], scale=cw)
                conv_tail(self.hH)

            def stage_z(self):
                n2, w, nt2 = self.n2, self.w, self.nt2
                self.e0p = spool.tile([H, 2 * MMT], bf16, name="e0p", tag="e0p")
                self.e12p = spool.tile([H, 4 * MMT], bf16, name="e12p",
                                       tag="e12p")
                e12pv = self.e12p.rearrange("p (t two w) -> p t two w",
                                            two=2, w=w)
                for m in range(3):
                    for off, n in self.halves:
                        hL, hR = self.hLR(off, n)
                        zk = [self.hp2[:, off:off + n], hL, hR,
                              self.u2[:, off:off + n]]
                        for k in (1, 2, 3, 0):
                            nc.tensor.matmul(self.pbs[m][:, off:off + n],
                                             wz[:, k, m, :], zk[k],
                                             start=(k == 1), stop=(k == 0))
                nc.scalar.activation(self.e0p[:, :n2], self.pbs[0][:, :n2],
                                     AF.Exp, bias=bv[:, 5:6])
                nc.scalar.activation(e12pv[:, :nt2, 0, :], self.pbs[1][:, :n2],
                                     AF.Exp, bias=bv[:, 6:7])
                nc.scalar.activation(e12pv[:, :nt2, 1, :], self.pbs[2][:, :n2],
                                     AF.Exp, bias=bv[:, 7:8])

            def post1(self):
                n2, w, nt2 = self.n2, self.w, self.nt2
                e12pv = self.e12p.rearrange("p (t two w) -> p t two w",
                                            two=2, w=w)
                self.p12 = spool.tile([H, 4 * MMT], bf16, name="p12", tag="p12",
                                      bufs=2)
                nc.vector.tensor_tensor(self.p12[:, :2 * n2], self.e12p[:, :2 * n2],
                                        self.cb_pair, OP.mult)
                self.p0 = spool.tile([H, 2 * MMT], bf16, name="p0", tag="p0",
                                     bufs=2)
                nc.vector.tensor_tensor(self.p0[:, :n2], self.e0p[:, :n2],
                                        self.hH, OP.mult)
                self.s1 = spool.tile([H, 2 * MMT], bf16, name="s1", tag="s1",
                                     bufs=2)
                nc.vector.tensor_tensor(self.s1[:, :n2], self.e0p[:, :n2],
                                        e12pv[:, :nt2, 0, :], OP.add)
                self.sf = spool.tile([H, 2 * MMT], f32, name="sf", tag="sf",
                                     bufs=2)
                nc.vector.scalar_tensor_tensor(self.sf[:, :n2], self.s1[:, :n2],
                                               1.0, e12pv[:, :nt2, 1, :],
                                               OP.add, OP.add)

            def post2(self):
                n2, w, nt2 = self.n2, self.w, self.nt2
                self.rcp = spool.tile([H, 2 * MMT], f32, name="rcp", tag="rcp",
                                      bufs=2)
                nc.vector.reciprocal_approx_fast(self.rcp[:, :n2],
                                                 self.sf[:, :n2])
                p12v = self.p12.rearrange("p (t two w) -> p t two w", two=2, w=w)
                self.bb = spool.tile([H, 2 * MMT], bf16, name="bb", tag="bb",
                                     bufs=2)
                bbv = self.bb.rearrange("p (t w) -> p t w", w=w)
                nc.vector.tensor_tensor(bbv[:, :nt2, :], p12v[:, :nt2, 0, :],
                                        p12v[:, :nt2, 1, :], OP.add)

            def post3(self):
                n2 = self.n2
                aa = spool.tile([H, 2 * MMT], bf16, name="aa", tag="aa", bufs=2)
                nc.vector.tensor_tensor(aa[:, :n2], self.p0[:, :n2], self.u,
                                        OP.add)
                num = spool.tile([H, 2 * MMT], bf16, name="num", tag="num",
                                 bufs=2)
                nc.vector.tensor_tensor(num[:, :n2], aa[:, :n2],
                                        self.bb[:, :n2], OP.add)
                nc.vector.tensor_tensor(self.out_ap, num[:, :n2],
                                        self.rcp[:, :n2], OP.mult)

        pending = [None]

        def emit_pair(pb):
            prev = pending[0]
            pb.stage_u()
            if prev:
                prev.post1()
            pb.stage_r()
            if prev:
                prev.post2()
            pb.stage_h()
            if prev:
                prev.post3()
            pb.stage_z()
            pending[0] = pb

        def flush_pending():
            prev = pending[0]
            if prev:
                prev.post1()
                prev.post2()
                prev.post3()
                pending[0] = None

        def run_level(nj, w, ct_base, cbuf, obuf):
            """One level with nj parents of per-tree width w, in tile pairs."""
            if nj <= 2 * MMT:
                # this level's first pair reads child columns written by the
                # still-pending pair -- interleaving would deadlock the queues
                flush_pending()
            done = 0
            while done < nj:
                piece = min(2048, nj - done)
                ctt = ctpool.tile([FEAT, 2048], bf16, name="ctt", tag="ctt")
                nc.sync.dma_start(ctt[:, :piece],
                                  ct_d[:, ct_base + done:ct_base + done + piece])
                for s in range(0, piece, 2 * MMT):
                    s0 = done + s
                    n2 = min(2 * MMT, piece - s)
                    pb = PairBlock(cbuf, w, s0, n2, ctt[:, s:s + n2],
                                   obuf[:, s0:s0 + n2])
                    emit_pair(pb)
                done += piece

        # ================= phase A: per-chunk levels 9..5 =================
        for c in range(NCHUNK):
            # leaf level 9
            nleaf = TCH * 512  # 8192
            base9 = LOFF[9] + c * nleaf
            for hpiece in range(0, nleaf, 2048):
                ctt = ctpool.tile([FEAT, 2048], bf16, name="ctt", tag="ctt")
                nc.sync.dma_start(ctt[:], ct_d[:, base9 + hpiece:base9 + hpiece + 2048])
                for s in range(0, 2048, 2 * MMT):
                    pu = pp.tile([H, 2 * MMT], f32, name="pu", tag="pa")
                    for hh in range(2):
                        nc.tensor.matmul(pu[:, hh * MMT:(hh + 1) * MMT], wu[:],
                                         ctt[:, s + hh * MMT:s + (hh + 1) * MMT],
                                         start=True, stop=True)
                    dst = e9[:, hpiece + s:hpiece + s + 2 * MMT]
                    nc.scalar.activation(dst, pu[:], AF.Relu,
                                         bias=bv[:, 0:1], scale=cw)
                    if not collapsible:
                        nc.scalar.activation(dst, dst, AF.Relu, bias=cb, scale=cw)
                        nc.scalar.activation(dst, dst, AF.Relu, bias=cb, scale=cw)
                if collapsible and do_affine:
                    big = e9[:, hpiece:hpiece + 2048]
                    nc.vector.tensor_scalar(big, big, A, C, OP.mult, OP.add)
            # inner levels 8..5
            for j, (cbuf, obuf) in zip(
                    range(8, 4, -1),
                    [(e9, e8), (e8, e7), (e7, e6), (e6, None)]):
                nj = TCH * (2 ** j)
                if j == 5:
                    ob = emb5[:, c * 512:(c + 1) * 512]
                else:
                    ob = obuf[:, :nj]
                run_level(nj, 2 ** j, LOFF[j] + c * nj, cbuf[:], ob)

        # ================= phase B: levels 4..0, all trees =================
        # reuse dead phase-A buffers for the tail levels
        e4 = e8[:, :2048]
        e3 = e7[:, :1024]
        e2 = e6[:, :512]
        e1 = e8[:, 2048:2048 + 256]
        e0f = epool.tile([H, TPC], f32, tag="e0f")
        e0t = e0f[:, :TPC]
        chain = [(emb5[:], e4), (e4, e3), (e3, e2), (e2, e1), (e1, e0t)]
        for j, (cbap, ob) in zip(range(4, -1, -1), chain):
            nj = TPC * (2 ** j)
            run_level(nj, 2 ** j, LOFF[j], cbap, ob)
        flush_pending()

        # ================= output transpose + store =================
        pt = pp.tile([H, H], f32, name="pt", tag="pa")
        nc.tensor.matmul(pt[:], e0t, idt[:], is_transpose=True, start=True, stop=True)
        osb = spool.tile([H, H], f32, name="osb", tag="osb")
        nc.vector.tensor_copy(osb[:], pt[:])
        nc.sync.dma_start(out_d[:], osb[:])

    nc.compile()
    if not nc.is_finalized():
        nc.finalize()
    return nc


def _prepare(inputs):
    contents = np.ascontiguousarray(np.asarray(inputs["contents"], np.float32))
    W_u = np.asarray(inputs["W_u"], np.float32)
    b_u = np.asarray(inputs["b_u"], np.float32)
    W_h = np.asarray(inputs["W_h"], np.float32)
    b_h = np.asarray(inputs["b_h"], np.float32)
    W_z = np.asarray(inputs["W_z"], np.float32)
    b_z = np.asarray(inputs["b_z"], np.float32)
    W_r = np.asarray(inputs["W_r"], np.float32)
    b_r = np.asarray(inputs["b_r"], np.float32)
    cw = float(np.asarray(inputs["conv_w"]).reshape(-1)[0])
    cb = float(np.asarray(inputs["conv_b"]).reshape(-1)[0])

    # per-core feature-major contents, level-major columns, bit-reversed
    # per-tree node order within each level
    cts = np.empty((NCORES, FEAT, NPC), np.float32)
    col = 0
    for j in range(L):
        n = TPC * 2 ** j
        blk = contents[OFF[j]:OFF[j + 1]].reshape(NCORES, TPC, 2 ** j, FEAT)
        if j > 0:
            blk = blk[:, :, _bitrev_perm(j), :]
        blk = blk.reshape(NCORES, n, FEAT)
        cts[:, :, col:col + n] = blk.transpose(0, 2, 1)
        col += n

    wr_np = np.ascontiguousarray(W_r.reshape(3, H, 3, H).transpose(1, 0, 2, 3))
    wh_np = np.ascontiguousarray((0.5 * W_h).reshape(3, H, H).transpose(1, 0, 2))
    # z-diff weights: Wd[k, :, m, :] = W_z[k,:,m,:] - W_z[k,:,3,:] for m=0..2
    wz4 = W_z.reshape(4, H, 4, H)
    wzd = np.ascontiguousarray(
        (wz4[:, :, 0:3, :] - wz4[:, :, 3:4, :]).transpose(1, 0, 2, 3))

    bvec = np.zeros((H, 8), np.float32)
    bvec[:, 0] = cw * b_u + cb
    bvec[:, 1:4] = 0.5 * b_r.reshape(3, H).T
    bvec[:, 4] = cw * b_h + cb
    bz4 = b_z.reshape(4, H)
    bvec[:, 5:8] = (bz4[0:3] - bz4[3:4]).T

    import ml_dtypes

    bf = ml_dtypes.bfloat16
    common = {
        "wu": np.ascontiguousarray(W_u).astype(bf),
        "wr": wr_np.astype(bf), "wh": wh_np.astype(bf), "wz": wzd.astype(bf),
        "bvec": bvec,
        "ident": np.eye(H, dtype=np.float32),
    }
    in_maps = [dict(common, ct=np.ascontiguousarray(cts[c]).astype(bf))
               for c in range(NCORES)]
    return in_maps, cw, cb


def kernel(**inputs):
    children = np.asarray(inputs["children"])
    cw = float(np.asarray(inputs["conv_w"]).reshape(-1)[0])
    cb = float(np.asarray(inputs["conv_b"]).reshape(-1)[0])
    collapsible = (cw >= 0.0) and (cb >= 0.0)
    if not _children_canonical(children):
        args = {k: np.asarray(v) for k, v in inputs.items()}
        return _numpy_fallback(**args)

    from concourse.bass_utils import run_bass_kernel_spmd

    A = cw * cw
    C = cw * cb + cb
    do_affine = not (A == 1.0 and C == 0.0)

    key = (cw, cb, collapsible, do_affine)
    if key not in _CACHE:
        _CACHE[key] = _build(cw, cb, collapsible, do_affine, A, C)
    nc = _CACHE[key]

    in_maps, _, _ = _prepare(inputs)
    res = run_bass_kernel_spmd(nc, in_maps, list(range(NCORES)))
    outs = [res.results[c]["out"] for c in range(NCORES)]
    return np.ascontiguousarray(np.concatenate(outs, axis=0).astype(np.float32))


if __name__ == "__main__":
    rng = np.random.default_rng(0)
    print("kernel module loaded")


# revision 13
# speedup vs baseline: 1.5675x; 1.1350x over previous
"""Trainium2 Bass kernel for nn_GRNNTransformGated (recursive tree GRNN over
1024 independent 10-level binary jets).

Strategy (v2):
  - Data-parallel over jets: 8 cores x 128 trees each.
  - BIT-REVERSED per-tree node layout: storing level j in bit-reversed local
    order makes the two children of parent position q land at positions q and
    q + 2^j of the level below -- every child "gather" becomes two contiguous
    stride-1 slices, so all DVE ops run in packed bf16 2x mode.
  - Softmax shift-invariance: z gates computed as differences d_m = z_m - z_3,
    so the z matmul is 4Hx3H (12 matmuls) instead of 4Hx4H (16), only 3 exps,
    e3 == 1 (numerator gets +u, denominator gets +1).
  - Feature-major layout [128 channels (partitions), nodes (free)]; weight
    blocks stationary, 25 matmuls per 512-node tile.
  - 8 separate PSUM banks (pu, pr0-2, ph, pz0-2) so consecutive tiles overlap
    with only early-read WAR dependencies -- keeps the PE dense and the HAM
    clock-gate warm (K=8/8).
  - conv_chain collapses: for w>0, b>=0, f(f(f(x))) = w^2*relu(w*x+b) + (w*b+b).
  - sigmoid via tanh; the 0.5 is folded into W_h on the host.
  - Per-tree interleaved temporaries (t01, e12, p12) let pairs of elementwise
    ops fuse into single FD=1024 stride-1 instructions.
"""

import sys

for _p in ("/opt/trn_rl_repo", "/root/.axon_site/_ro/trn_rl_repo"):
    if _p not in sys.path:
        sys.path.insert(0, _p)

import numpy as np

B = 1024
L = 10
H = 128
FEAT = 7
NCORES = 8
TPC = B // NCORES          # trees per core = 128
TCH = 16                   # trees per chunk
NCHUNK = TPC // TCH        # 8 chunks
NPC = TPC * (2 ** L - 1)   # nodes per core = 130944
LOFF = [TPC * (2 ** j - 1) for j in range(L + 1)]  # level offsets in per-core ct
LEVEL_SIZES = [B * 2 ** j for j in range(L)]
OFF = np.concatenate([[0], np.cumsum(LEVEL_SIZES)]).astype(int)
INNER = LEVEL_SIZES[:-1]
COFF = np.concatenate([[0], np.cumsum(INNER)]).astype(int)

MMT = 512  # matmul node-tile size

_CACHE = {}


def _children_canonical(children):
    for j in range(L - 1):
        n = INNER[j]
        blk = children[COFF[j]:COFF[j + 1]]
        base = 2 * np.arange(n, dtype=np.int64)
        if not (np.array_equal(blk[:, 0], base) and np.array_equal(blk[:, 1], base + 1)):
            return False
    return True


def _numpy_fallback(contents, children, W_u, b_u, W_h, b_h, W_z, b_z, W_r, b_r,
                    conv_w, conv_b):
    w, b = float(conv_w[0]), float(conv_b[0])

    def conv_chain(x):
        for _ in range(3):
            x = np.maximum(w * x + b, 0.0)
        return x

    def sigmoid(x):
        return 1.0 / (1.0 + np.exp(-x))

    emb = None
    for j in reversed(range(L)):
        c = contents[OFF[j]:OFF[j + 1]]
        u = conv_chain(c @ W_u + b_u)
        if j == L - 1:
            emb = u
            continue
        ch = children[COFF[j]:COFF[j + 1]]
        h_L = emb[ch[:, 0]]
        h_R = emb[ch[:, 1]]
        hhu = np.concatenate([h_L, h_R, u], axis=1)
        r = sigmoid(hhu @ W_r + b_r)
        h_H = conv_chain((r * hhu) @ W_h + b_h)
        z = np.concatenate([h_H, hhu], axis=1) @ W_z + b_z
        zs = np.stack([z[:, :H], z[:, H:2 * H], z[:, 2 * H:3 * H], z[:, 3 * H:]], axis=-1)
        zs = zs - zs.max(axis=-1, keepdims=True)
        e = np.exp(zs)
        g = e / e.sum(axis=-1, keepdims=True)
        emb = g[..., 0] * h_H + g[..., 1] * h_L + g[..., 2] * h_R + g[..., 3] * u
    return emb.reshape(B, -1).astype(np.float32)


def _bitrev_perm(j):
    """perm[q] = bit-reverse of q over j bits."""
    if j == 0:
        return np.zeros(1, dtype=np.int64)
    return (
        np.arange(2 ** j, dtype=np.int64)
        .reshape((2,) * j)
        .transpose(tuple(reversed(range(j))))
        .ravel()
    )


def _build(cw, cb, collapsible, do_affine, A, C):
    from contextlib import ExitStack

    from concourse import bacc, bass, mybir, tile

    f32 = mybir.dt.float32
    bf16 = mybir.dt.bfloat16
    AF = mybir.ActivationFunctionType
    OP = mybir.AluOpType

    nc = bacc.Bacc()

    ct_d = nc.declare_dram_parameter("ct", [FEAT, NPC], bf16, isOutput=False)
    wu_d = nc.declare_dram_parameter("wu", [FEAT, H], bf16, isOutput=False)
    wr_d = nc.declare_dram_parameter("wr", [H, 3, 3, H], bf16, isOutput=False)
    wh_d = nc.declare_dram_parameter("wh", [H, 3, H], bf16, isOutput=False)
    wz_d = nc.declare_dram_parameter("wz", [H, 4, 3, H], bf16, isOutput=False)
    bv_d = nc.declare_dram_parameter("bvec", [H, 8], f32, isOutput=False)
    id_d = nc.declare_dram_parameter("ident", [H, H], f32, isOutput=False)
    out_d = nc.declare_dram_parameter("out", [TPC, H], f32, isOutput=True)

    with ExitStack() as ctx:
        tc = ctx.enter_context(tile.TileContext(nc))
        wpool = ctx.enter_context(tc.tile_pool(name="wts", bufs=1))
        epool = ctx.enter_context(tc.tile_pool(name="emb", bufs=1))
        ctpool = ctx.enter_context(tc.tile_pool(name="ct", bufs=3))
        spool = ctx.enter_context(tc.tile_pool(name="tmp", bufs=3))
        pp = ctx.enter_context(tc.tile_pool(name="ps", bufs=1, space="PSUM"))

        wu = wpool.tile([FEAT, H], bf16, tag="wu")
        wr = wpool.tile([H, 3, 3, H], bf16, tag="wr")
        wh = wpool.tile([H, 3, H], bf16, tag="wh")
        wz = wpool.tile([H, 4, 3, H], bf16, tag="wz")
        bv = wpool.tile([H, 8], f32, tag="bv")
        idt = wpool.tile([H, H], f32, tag="idt")
        nc.sync.dma_start(wu[:], wu_d[:])
        nc.sync.dma_start(wr[:], wr_d[:])
        nc.sync.dma_start(wh[:], wh_d[:])
        nc.sync.dma_start(wz[:], wz_d[:])
        nc.sync.dma_start(bv[:], bv_d[:])
        nc.sync.dma_start(idt[:], id_d[:])

        # emb level buffers (e9/e8 hold one chunk; e7f/e6f/emb5 all trees)
        e9 = epool.tile([H, TCH * 512], bf16, tag="e9")     # 8192
        e8 = epool.tile([H, TCH * 256], bf16, tag="e8")     # 4096
        emb5 = epool.tile([H, TPC * 32], bf16, tag="emb5")  # 4096 (all trees)

        def conv_tail(dst):
            if collapsible:
                if do_affine:
                    nc.vector.tensor_scalar(dst, dst, A, C, OP.mult, OP.add)
            else:
                nc.scalar.activation(dst, dst, AF.Relu, bias=cb, scale=cw)
                nc.scalar.activation(dst, dst, AF.Relu, bias=cb, scale=cw)

        class PairBlock:
            """One 1024-node tile pair: matmuls run per 512 half (PSUM bank
            limit), activations and DVE ops run pair-wide (FD=1024/2048).
            PSUM: pa (2 banks, u only), pb0..pb2 (2 banks each; pb0 also holds
            the h and z0 accumulations -- their WARs are implied by true deps).
            Post-softmax work is split into chunks that the driver interleaves
            with the NEXT pair's front stages, so every engine queue stays in
            data-ready order."""

            def __init__(self, cbuf, w, s0, n2, ct_ap, out_ap):
                self.w, self.s0, self.n2 = w, s0, n2
                self.ct_ap = ct_ap
                self.out_ap = out_ap
                self.nt2 = n2 // w
                self.t0 = s0 // w
                self.cb4 = cbuf.rearrange("p (t two w) -> p t two w", two=2, w=w)
                self.cb_pair = cbuf[:, 2 * s0:2 * s0 + 2 * n2]
                self.halves = []
                off = 0
                while off < n2:
                    n = min(MMT, n2 - off)
                    self.halves.append((off, n))
                    off += n

            def hLR(self, off, n):
                tt0 = self.t0 + off // self.w
                ntt = n // self.w
                return (self.cb4[:, tt0:tt0 + ntt, 0, :],
                        self.cb4[:, tt0:tt0 + ntt, 1, :])

            def stage_u(self):
                n2 = self.n2
                self.pa = pp.tile([H, 2 * MMT], f32, name="pa", tag="pa")
                for off, n in self.halves:
                    nc.tensor.matmul(self.pa[:, off:off + n], wu[:],
                                     self.ct_ap[:, off:off + n],
                                     start=True, stop=True)
                self.u2 = spool.tile([H, 2 * MMT], bf16, name="u2", tag="u2")
                self.u = self.u2[:, :n2]
                nc.scalar.activation(self.u, self.pa[:, :n2], AF.Relu,
                                     bias=bv[:, 0:1], scale=cw)
                conv_tail(self.u)

            def stage_r(self):
                n2, w, nt2 = self.n2, self.w, self.nt2
                self.pbs = [pp.tile([H, 2 * MMT], f32, name=f"pb{m}", tag=f"pb{m}")
                            for m in range(3)]
                for m in range(3):
                    for off, n in self.halves:
                        hL, hR = self.hLR(off, n)
                        rhs = [hL, hR, self.u2[:, off:off + n]]
                        for k in range(3):
                            nc.tensor.matmul(self.pbs[m][:, off:off + n],
                                             wr[:, k, m, :], rhs[k],
                                             start=(k == 0), stop=(k == 2))
                self.t01 = spool.tile([H, 4 * MMT], bf16, name="t01", tag="t01",
                                      bufs=2)
                t01v = self.t01.rearrange("p (t two w) -> p t two w", two=2, w=w)
                self.t2 = spool.tile([H, 2 * MMT], bf16, name="t2", tag="t2",
                                     bufs=2)
                for m in range(2):
                    nc.scalar.activation(t01v[:, :nt2, m, :], self.pbs[m][:, :n2],
                                         AF.Tanh, bias=bv[:, 1 + m:2 + m],
                                         scale=0.5)
                nc.scalar.activation(self.t2[:, :n2], self.pbs[2][:, :n2],
                                     AF.Tanh, bias=bv[:, 3:4], scale=0.5)

            def stage_h(self):
                # W_h-folded: h_psum = sum_k wh[k]@(t_k*x_k) + wh[k]@x_k
                # (the GRU 0.5*(t+1) is the half-sum of those two terms; the
                # 0.5 is pre-folded into wh on the host)
                n2, w = self.n2, self.w
                th01 = spool.tile([H, 4 * MMT], bf16, name="th01", tag="th01",
                                  bufs=2)
                nc.vector.tensor_tensor(th01[:, :2 * n2], self.t01[:, :2 * n2],
                                        self.cb_pair, OP.mult)
                th2 = spool.tile([H, 2 * MMT], bf16, name="th2", tag="th2",
                                 bufs=2)
                nc.vector.tensor_tensor(th2[:, :n2], self.t2[:, :n2], self.u,
                                        OP.mult)
                th01v = th01.rearrange("p (t two w) -> p t two w", two=2, w=w)
                ph = self.pbs[0]
                for off, n in self.halves:
                    tt0 = off // w
                    ntt = n // w
                    hL, hR = self.hLR(off, n)
                    terms = [
                        (wh[:, 0, :], th01v[:, tt0:tt0 + ntt, 0, :]),
                        (wh[:, 1, :], th01v[:, tt0:tt0 + ntt, 1, :]),
                        (wh[:, 2, :], th2[:, off:off + n]),
                        (wh[:, 0, :], hL),
                        (wh[:, 1, :], hR),
                        (wh[:, 2, :], self.u2[:, off:off + n]),
                    ]
                    for i, (wt, rhs) in enumerate(terms):
                        nc.tensor.matmul(ph[:, off:off + n], wt, rhs,
                                         start=(i == 0),
                                         stop=(i == len(terms) - 1))
                self.hp2 = spool.tile([H, 2 * MMT], bf16, name="hp2", tag="hp2")
                self.hH = self.hp2[:, :n2]
                nc.scalar.activation(self.hH, ph[:, :n2], AF.Relu,
                                     bias=bv[:, 4:5], scale=cw)
                conv_tail(self.hH)

            def stage_z(self):
                n2, w, nt2 = self.n2, self.w, self.nt2
                self.e0p = spool.tile([H, 2 * MMT], bf16, name="e0p", tag="e0p")
                self.e12p = spool.tile([H, 4 * MMT], bf16, name="e12p",
                                       tag="e12p")
                e12pv = self.e12p.rearrange("p (t two w) -> p t two w",
                                            two=2, w=w)
                for m in range(3):
                    for off, n in self.halves:
                        hL, hR = self.hLR(off, n)
                        zk = [self.hp2[:, off:off + n], hL, hR,
                              self.u2[:, off:off + n]]
                        for k in (1, 2, 3, 0):
                            nc.tensor.matmul(self.pbs[m][:, off:off + n],
                                             wz[:, k, m, :], zk[k],
                                             start=(k == 1), stop=(k == 0))
                nc.scalar.activation(self.e0p[:, :n2], self.pbs[0][:, :n2],
                                     AF.Exp, bias=bv[:, 5:6])
                nc.scalar.activation(e12pv[:, :nt2, 0, :], self.pbs[1][:, :n2],
                                     AF.Exp, bias=bv[:, 6:7])
                nc.scalar.activation(e12pv[:, :nt2, 1, :], self.pbs[2][:, :n2],
                                     AF.Exp, bias=bv[:, 7:8])

            def post1(self):
                n2, w, nt2 = self.n2, self.w, self.nt2
                e12pv = self.e12p.rearrange("p (t two w) -> p t two w",
                                            two=2, w=w)
                self.p12 = spool.tile([H, 4 * MMT], bf16, name="p12", tag="p12",
                                      bufs=2)
                nc.vector.tensor_tensor(self.p12[:, :2 * n2], self.e12p[:, :2 * n2],
                                        self.cb_pair, OP.mult)
                self.p0 = spool.tile([H, 2 * MMT], bf16, name="p0", tag="p0",
                                     bufs=2)
                nc.vector.tensor_tensor(self.p0[:, :n2], self.e0p[:, :n2],
                                        self.hH, OP.mult)
                self.s1 = spool.tile([H, 2 * MMT], bf16, name="s1", tag="s1",
                                     bufs=2)
                nc.vector.tensor_tensor(self.s1[:, :n2], self.e0p[:, :n2],
                                        e12pv[:, :nt2, 0, :], OP.add)
                self.sf = spool.tile([H, 2 * MMT], f32, name="sf", tag="sf",
                                     bufs=2)
                nc.vector.scalar_tensor_tensor(self.sf[:, :n2], self.s1[:, :n2],
                                               1.0, e12pv[:, :nt2, 1, :],
                                               OP.add, OP.add)

            def post2(self):
                n2, w, nt2 = self.n2, self.w, self.nt2
                self.rcp = spool.tile([H, 2 * MMT], f32, name="rcp", tag="rcp",
                                      bufs=2)
                nc.vector.reciprocal_approx_fast(self.rcp[:, :n2],
                                                 self.sf[:, :n2])
                p12v = self.p12.rearrange("p (t two w) -> p t two w", two=2, w=w)
                self.bb = spool.tile([H, 2 * MMT], bf16, name="bb", tag="bb",
                                     bufs=2)
                bbv = self.bb.rearrange("p (t w) -> p t w", w=w)
                nc.vector.tensor_tensor(bbv[:, :nt2, :], p12v[:, :nt2, 0, :],
                                        p12v[:, :nt2, 1, :], OP.add)

            def post3(self):
                n2 = self.n2
                aa = spool.tile([H, 2 * MMT], bf16, name="aa", tag="aa", bufs=2)
                nc.vector.tensor_tensor(aa[:, :n2], self.p0[:, :n2], self.u,
                                        OP.add)
                num = spool.tile([H, 2 * MMT], bf16, name="num", tag="num",
                                 bufs=2)
                nc.vector.tensor_tensor(num[:, :n2], aa[:, :n2],
                                        self.bb[:, :n2], OP.add)
                nc.vector.tensor_tensor(self.out_ap, num[:, :n2],
                                        self.rcp[:, :n2], OP.mult)

        pending = [None]

        def emit_pair(pb):
            # software pipeline: the previous pair's z matmuls + exps are
            # deferred into this cycle so relu_u(new) precedes exps(prev) in
            # the in-order scalar queue; post chunks trail one more step
            prev = pending[0]
            pb.stage_u()
            if prev:
                prev.stage_z()
            pb.stage_r()
            if prev:
                prev.post1()
            pb.stage_h()
            if prev:
                prev.post2()
                prev.post3()
            pending[0] = pb

        def flush_pending():
            prev = pending[0]
            if prev:
                prev.stage_z()
                prev.post1()
                prev.post2()
                prev.post3()
                pending[0] = None

        def run_span(w, ct_base, cbuf, obuf, lo, hi):
            """Emit pairs covering parent columns [lo, hi) of a level whose
            per-tree width is w. The caller guarantees the still-pending
            pair's output does not overlap this span's child columns."""
            done = lo
            while done < hi:
                piece = min(2048, hi - done)
                ctt = ctpool.tile([FEAT, 2048], bf16, name="ctt", tag="ctt")
                nc.sync.dma_start(ctt[:, :piece],
                                  ct_d[:, ct_base + done:ct_base + done + piece])
                for s in range(0, piece, 2 * MMT):
                    s0 = done + s
                    n2 = min(2 * MMT, piece - s)
                    pb = PairBlock(cbuf, w, s0, n2, ctt[:, s:s + n2],
                                   obuf[:, s0:s0 + n2])
                    emit_pair(pb)
                done += piece

        # ================= phase A =================
        # per-chunk: leaves -> level 8 -> level 7 (into the full-size e7f);
        # then levels 6 and 5 run globally over all chunks so every level has
        # enough pairs to keep the software pipeline full (no flushes).
        e7f = epool.tile([H, TPC * 128], bf16, tag="e7f")   # 16384
        e6f = epool.tile([H, TPC * 64], bf16, tag="e6f")    # 8192
        for c in range(NCHUNK):
            # leaf level 9
            nleaf = TCH * 512  # 8192
            base9 = LOFF[9] + c * nleaf
            for hpiece in range(0, nleaf, 2048):
                ctt = ctpool.tile([FEAT, 2048], bf16, name="ctt", tag="ctt")
                nc.sync.dma_start(ctt[:], ct_d[:, base9 + hpiece:base9 + hpiece + 2048])
                for s in range(0, 2048, 2 * MMT):
                    pu = pp.tile([H, 2 * MMT], f32, name="pu", tag="pa")
                    for hh in range(2):
                        nc.tensor.matmul(pu[:, hh * MMT:(hh + 1) * MMT], wu[:],
                                         ctt[:, s + hh * MMT:s + (hh + 1) * MMT],
                                         start=True, stop=True)
                    dst = e9[:, hpiece + s:hpiece + s + 2 * MMT]
                    nc.scalar.activation(dst, pu[:], AF.Relu,
                                         bias=bv[:, 0:1], scale=cw)
                    if not collapsible:
                        nc.scalar.activation(dst, dst, AF.Relu, bias=cb, scale=cw)
                        nc.scalar.activation(dst, dst, AF.Relu, bias=cb, scale=cw)
                if collapsible and do_affine:
                    big = e9[:, hpiece:hpiece + 2048]
                    nc.vector.tensor_scalar(big, big, A, C, OP.mult, OP.add)
            # level 8 (per chunk) and level 7 (per chunk, into e7f)
            nj8 = TCH * 256
            run_span(256, LOFF[8] + c * nj8, e9[:], e8[:, :nj8], 0, nj8)
            nj7 = TCH * 128
            ob7 = e7f[:, c * nj7:(c + 1) * nj7]
            run_span(128, LOFF[7] + c * nj7, e8[:], ob7, 0, nj7)
        # global levels 6 and 5
        run_span(64, LOFF[6], e7f[:], e6f[:], 0, TPC * 64)
        run_span(32, LOFF[5], e6f[:], emb5[:], 0, TPC * 32)

        # ================= phase B: levels 4..0, all trees =================
        # split into independent per-tree-group chains so the tiny levels
        # pipeline against each other instead of serializing
        e4 = e8[:, :2048]
        e3 = e7f[:, :1024]
        e2 = e6f[:, :512]
        e1 = e8[:, 2048:2048 + 256]
        e0f = epool.tile([H, TPC], f32, tag="e0f")
        e0t = e0f[:, :TPC]
        for j, cbap, ob, nsplit in (
                (4, emb5[:], e4, 4), (3, e4, e3, 4),
                (2, e3, e2, 2), (1, e2, e1, 2)):
            nj = TPC * (2 ** j)
            for q in range(nsplit):
                span = nj // nsplit
                run_span(2 ** j, LOFF[j], cbap, ob, q * span, (q + 1) * span)
        # level 0 reads both halves of e1 incl. the pending pair's output
        flush_pending()
        run_span(1, LOFF[0], e1, e0t, 0, TPC)
        flush_pending()

        # ================= output transpose + store =================
        pt = pp.tile([H, H], f32, name="pt", tag="pa")
        nc.tensor.matmul(pt[:], e0t, idt[:], is_transpose=True, start=True, stop=True)
        osb = spool.tile([H, H], f32, name="osb", tag="osb")
        nc.vector.tensor_copy(osb[:], pt[:])
        nc.sync.dma_start(out_d[:], osb[:])

    nc.compile()
    if not nc.is_finalized():
        nc.finalize()
    return nc


def _prepare(inputs):
    contents = np.ascontiguousarray(np.asarray(inputs["contents"], np.float32))
    W_u = np.asarray(inputs["W_u"], np.float32)
    b_u = np.asarray(inputs["b_u"], np.float32)
    W_h = np.asarray(inputs["W_h"], np.float32)
    b_h = np.asarray(inputs["b_h"], np.float32)
    W_z = np.asarray(inputs["W_z"], np.float32)
    b_z = np.asarray(inputs["b_z"], np.float32)
    W_r = np.asarray(inputs["W_r"], np.float32)
    b_r = np.asarray(inputs["b_r"], np.float32)
    cw = float(np.asarray(inputs["conv_w"]).reshape(-1)[0])
    cb = float(np.asarray(inputs["conv_b"]).reshape(-1)[0])

    # per-core feature-major contents, level-major columns, bit-reversed
    # per-tree node order within each level
    cts = np.empty((NCORES, FEAT, NPC), np.float32)
    col = 0
    for j in range(L):
        n = TPC * 2 ** j
        blk = contents[OFF[j]:OFF[j + 1]].reshape(NCORES, TPC, 2 ** j, FEAT)
        if j > 0:
            blk = blk[:, :, _bitrev_perm(j), :]
        blk = blk.reshape(NCORES, n, FEAT)
        cts[:, :, col:col + n] = blk.transpose(0, 2, 1)
        col += n

    wr_np = np.ascontiguousarray(W_r.reshape(3, H, 3, H).transpose(1, 0, 2, 3))
    wh_np = np.ascontiguousarray((0.5 * W_h).reshape(3, H, H).transpose(1, 0, 2))
    # z-diff weights: Wd[k, :, m, :] = W_z[k,:,m,:] - W_z[k,:,3,:] for m=0..2
    wz4 = W_z.reshape(4, H, 4, H)
    wzd = np.ascontiguousarray(
        (wz4[:, :, 0:3, :] - wz4[:, :, 3:4, :]).transpose(1, 0, 2, 3))

    bvec = np.zeros((H, 8), np.float32)
    bvec[:, 0] = cw * b_u + cb
    bvec[:, 1:4] = 0.5 * b_r.reshape(3, H).T
    bvec[:, 4] = cw * b_h + cb
    bz4 = b_z.reshape(4, H)
    bvec[:, 5:8] = (bz4[0:3] - bz4[3:4]).T

    import ml_dtypes

    bf = ml_dtypes.bfloat16
    common = {
        "wu": np.ascontiguousarray(W_u).astype(bf),
        "wr": wr_np.astype(bf), "wh": wh_np.astype(bf), "wz": wzd.astype(bf),
        "bvec": bvec,
        "ident": np.eye(H, dtype=np.float32),
    }
    in_maps = [dict(common, ct=np.ascontiguousarray(cts[c]).astype(bf))
               for c in range(NCORES)]
    return in_maps, cw, cb


def kernel(**inputs):
    children = np.asarray(inputs["children"])
    cw = float(np.asarray(inputs["conv_w"]).reshape(-1)[0])
    cb = float(np.asarray(inputs["conv_b"]).reshape(-1)[0])
    collapsible = (cw >= 0.0) and (cb >= 0.0)
    if not _children_canonical(children):
        args = {k: np.asarray(v) for k, v in inputs.items()}
        return _numpy_fallback(**args)

    from concourse.bass_utils import run_bass_kernel_spmd

    A = cw * cw
    C = cw * cb + cb
    do_affine = not (A == 1.0 and C == 0.0)

    key = (cw, cb, collapsible, do_affine)
    if key not in _CACHE:
        _CACHE[key] = _build(cw, cb, collapsible, do_affine, A, C)
    nc = _CACHE[key]

    in_maps, _, _ = _prepare(inputs)
    res = run_bass_kernel_spmd(nc, in_maps, list(range(NCORES)))
    outs = [res.results[c]["out"] for c in range(NCORES)]
    return np.ascontiguousarray(np.concatenate(outs, axis=0).astype(np.float32))


if __name__ == "__main__":
    rng = np.random.default_rng(0)
    print("kernel module loaded")
